# revision 18
# baseline (speedup 1.0000x reference)
"""Trainium2 Bass kernel for nn_Encoder_38259568672815 (ViT-style encoder).

Strategy: data-parallel over batch (16 images -> 8 cores x 2 images).
On-chip layout: feature-major residual stream [D on partitions, tokens free].
bf16 matmul operands + residual storage, fp32 PSUM accumulation and LN /
softmax statistics.

Self-contained: hardcodes all shapes; host work is limited to layout
permutations (im2col, weight casts, final transpose) and sharding.
"""
from contextlib import ExitStack
import os
SKIP = set(os.environ.get("KSKIP", "").split(","))

import numpy as np
import ml_dtypes

import concourse.bass as bass
import concourse.tile as tile
import concourse.mybir as mybir
from concourse import bacc
from concourse.masks import make_identity
from concourse.bass_utils import run_bass_kernel_spmd

F32 = mybir.dt.float32
BF16 = mybir.dt.bfloat16
F16 = mybir.dt.float16
AF = mybir.ActivationFunctionType

B, C, IMG, P = 16, 3, 384, 16
D, NH, DK, L, FF = 768, 12, 64, 6, 3072
S = (IMG // P) ** 2          # 576 tokens per image
NI = 2                       # images per core
T = NI * S                   # 1152 token columns per core
DT = D // 128                # 6 d-tiles
FT = FF // 128               # 24 f-tiles
ST = (S + 127) // 128        # 5 token tiles per image (last = 64)
TCH = 3                      # token chunks of 384 over T
QCH = 2                      # q chunks of 288 per image
QW = S // QCH                # 288
NCORES = 8


def _stiles(img):
    """(kt, row0, ss) k-token tiles for one image."""
    out = []
    for kt in range(ST):
        ss = min(128, S - kt * 128)
        out.append((kt, img * S + kt * 128, ss))
    return out


def build_kernel(n_layers=L):
    nc = bacc.Bacc()

    # ---- DRAM tensors ----
    xp = nc.dram_tensor("xp", [NI, D, S], BF16, kind="ExternalInput")
    wck = nc.dram_tensor("wck", [D, D], BF16, kind="ExternalInput")
    cb = nc.dram_tensor("cb", [D], F32, kind="ExternalInput")
    pef = nc.dram_tensor("pef", [D, S], F32, kind="ExternalInput")
    wq = nc.dram_tensor("wq", [L, D, D], BF16, kind="ExternalInput")
    wk = nc.dram_tensor("wk", [L, D, D], BF16, kind="ExternalInput")
    wv = nc.dram_tensor("wv", [L, D, D], BF16, kind="ExternalInput")
    wh = nc.dram_tensor("wh", [L, D, D], BF16, kind="ExternalInput")
    whb = nc.dram_tensor("whb", [L, D], F32, kind="ExternalInput")
    ln2s = nc.dram_tensor("ln2s", [L, D], F32, kind="ExternalInput")
    ln2b = nc.dram_tensor("ln2b", [L, D], F32, kind="ExternalInput")
    w1 = nc.dram_tensor("w1", [L, FT, 128, DT, 128], BF16, kind="ExternalInput")
    b1 = nc.dram_tensor("b1", [L, FF], F32, kind="ExternalInput")
    w2 = nc.dram_tensor("w2", [L, FF, D], BF16, kind="ExternalInput")
    b2 = nc.dram_tensor("b2", [L, D], F32, kind="ExternalInput")
    lnfs = nc.dram_tensor("lnfs", [D], F32, kind="ExternalInput")
    lnfb = nc.dram_tensor("lnfb", [D], F32, kind="ExternalInput")
    out = nc.dram_tensor("out", [NI, D, S], F32, kind="ExternalOutput")
    scratch = nc.dram_tensor("scratch", [NI, S, D], BF16)

    with tile.TileContext(nc) as tc, ExitStack() as ctx, \
            nc.allow_low_precision(reason="bf16 residual stream by design"):
        xpool = ctx.enter_context(tc.tile_pool(name="x", bufs=1))
        x1pool = ctx.enter_context(tc.tile_pool(name="x1", bufs=1))
        consts = ctx.enter_context(tc.tile_pool(name="consts", bufs=1))
        biasp = ctx.enter_context(tc.tile_pool(name="biasp", bufs=2))

        ones_col = consts.tile([128, 1], BF16)
        nc.vector.memset(ones_col[:], 1.0)
        ones_f = consts.tile([128, 1], F32)
        nc.vector.memset(ones_f[:], 1.0)
        ones64 = consts.tile([128, 64], BF16)
        nc.vector.memset(ones64[:], 1.0)
        ones_row = consts.tile([1, 128], F32)
        nc.vector.memset(ones_row[:], 1.0)
        eps2 = consts.tile([1, 1], F32)
        nc.vector.memset(eps2[:], 1e-6)
        epsf = consts.tile([1, 1], F32)
        nc.vector.memset(epsf[:], 1e-12)
        ident = consts.tile([128, 128], BF16)
        make_identity(nc, ident[:])

        x_sb = xpool.tile([128, DT, T], F32)
        x1_sb = x1pool.tile([128, DT, T], F32)

        # ================= Phase A: conv patch embedding =================
        with tc.tile_pool(name="conv", bufs=1) as convp, \
             tc.tile_pool(name="cps", bufs=3, space="PSUM") as cps, \
             tc.tile_pool(name="emb", bufs=2) as embp:
            wck_sb = convp.tile([128, DT, D], BF16)
            nc.sync.dma_start(wck_sb[:], wck.rearrange("(t p) d -> p t d", p=128))
            cb_sb = convp.tile([128, DT], F32)
            nc.sync.dma_start(cb_sb[:], cb.rearrange("(t p) -> p t", p=128))
            xp_sb = convp.tile([128, NI, DT, S], BF16)
            nc.sync.dma_start(xp_sb[:], xp.rearrange("b (t p) s -> p b t s", p=128))
            for img in range(NI):
                emb_sb = embp.tile([128, DT, S], BF16)
                for dm in range(DT):
                    for ch in range(QCH):
                        ps = cps.tile([128, QW], F32)
                        for kt in range(DT):
                            nc.tensor.matmul(
                                ps[:],
                                wck_sb[:, kt, dm * 128:(dm + 1) * 128],
                                xp_sb[:, img, kt, ch * QW:(ch + 1) * QW],
                                start=(kt == 0), stop=(kt == DT - 1))
                        nc.scalar.activation(
                            emb_sb[:, dm, ch * QW:(ch + 1) * QW], ps[:],
                            AF.Tanh, bias=cb_sb[:, dm:dm + 1])
                # write d-major flat: flat[(d, s)] with d = t*128 + p
                nc.sync.dma_start(
                    scratch[img].rearrange("s d -> (s d)").rearrange(
                        "(t p s) -> p t s", p=128, s=S),
                    emb_sb[:])

        # ============ Phase B: reshape quirk + pos-enc -> x (bf16) ============
        with tc.tile_pool(name="htok", bufs=3) as hp, \
             tc.tile_pool(name="tps", bufs=4, space="PSUM") as tps, \
             tc.tile_pool(name="pe", bufs=1) as pep:
            pe_sb = pep.tile([128, DT, S], F32)
            nc.sync.dma_start(pe_sb[:], pef.rearrange("(t p) s -> p t s", p=128))
            for img in range(NI):
                for st in range(ST):
                    ss = min(128, S - st * 128)
                    h_sb = hp.tile([128, D], BF16)
                    nc.sync.dma_start(h_sb[:ss, :],
                                      scratch[img, st * 128:st * 128 + ss, :])
                    for dtile in range(DT):
                        pst = tps.tile([128, 128], BF16)
                        nc.tensor.transpose(
                            pst[:, 0:ss], h_sb[:ss, dtile * 128:(dtile + 1) * 128],
                            ident[0:ss, 0:ss])
                        nc.vector.tensor_add(
                            x_sb[:, dtile, img * S + st * 128: img * S + st * 128 + ss],
                            pst[:, 0:ss], pe_sb[:, dtile, st * 128:st * 128 + ss])

        # phase boundary: keep conv/permute pools from overlapping layer pools
        tc.strict_bb_all_engine_barrier()

        # layer-phase pools (opened after conv pools close to fit SBUF)
        xbfp = ctx.enter_context(tc.tile_pool(name="xbf", bufs=1))
        wqkv = ctx.enter_context(tc.tile_pool(name="wqkv", bufs=4))
        qkp = ctx.enter_context(tc.tile_pool(name="qk", bufs=2))
        vp = ctx.enter_context(tc.tile_pool(name="v", bufs=1))
        ep = ctx.enter_context(tc.tile_pool(name="E", bufs=2))
        hvp = ctx.enter_context(tc.tile_pool(name="hv", bufs=1))
        smallp = ctx.enter_context(tc.tile_pool(name="small", bufs=1))
        rsbp = ctx.enter_context(tc.tile_pool(name="rsb", bufs=3))
        ffw = ctx.enter_context(tc.tile_pool(name="ffw", bufs=4))
        gp = ctx.enter_context(tc.tile_pool(name="g", bufs=2))
        tmpp = ctx.enter_context(tc.tile_pool(name="tmp", bufs=2))

        # ================= Phase C: encoder layers =================
        cur, nxt = x_sb, x1_sb
        for li in range(n_layers):
            wq_sb = wqkv.tile([128, DT, D], BF16, tag="w4")
            nc.sync.dma_start(wq_sb[:], wq[li].rearrange("(t p) e -> p t e", p=128))
            wk_sb = wqkv.tile([128, DT, D], BF16, tag="w4")
            nc.sync.dma_start(wk_sb[:], wk[li].rearrange("(t p) e -> p t e", p=128))
            wv_sb = wqkv.tile([128, DT, D], BF16, tag="w4")
            nc.sync.dma_start(wv_sb[:], wv[li].rearrange("(t p) e -> p t e", p=128))
            whb_sb = biasp.tile([128, DT], F32, tag="whb")
            nc.sync.dma_start(whb_sb[:], whb[li].rearrange("(t p) -> p t", p=128))
            l2s_sb = biasp.tile([128, DT], F32, tag="l2s")
            nc.sync.dma_start(l2s_sb[:], ln2s[li].rearrange("(t p) -> p t", p=128))
            l2b_sb = biasp.tile([128, DT], F32, tag="l2b")
            nc.sync.dma_start(l2b_sb[:], ln2b[li].rearrange("(t p) -> p t", p=128))
            b1_sb = biasp.tile([128, FT], F32, tag="b1")
            nc.sync.dma_start(b1_sb[:], b1[li].rearrange("(t p) -> p t", p=128))
            b2_sb = biasp.tile([128, DT], F32, tag="b2")
            nc.sync.dma_start(b2_sb[:], b2[li].rearrange("(t p) -> p t", p=128))

            # ---- C0: bf16 cast of residual for matmul operands ----
            xc = xbfp.tile([128, DT, T], BF16, tag="xc")
            for dtile in range(DT):
                eng = nc.vector if dtile % 2 == 0 else nc.gpsimd
                eng.tensor_copy(xc[:, dtile, :], cur[:, dtile, :])

            wh_sb = wqkv.tile([128, DT, D], BF16, tag="w4")
            nc.sync.dma_start(wh_sb[:], wh[li].rearrange("(t p) e -> p t e", p=128))

            with tc.tile_pool(name="qps", bufs=2, space="PSUM") as qps, \
                 tc.tile_pool(name="sps", bufs=2, space="PSUM") as sps, \
                 tc.tile_pool(name="hps", bufs=2, space="PSUM") as hps, \
                 tc.tile_pool(name="dps", bufs=2, space="PSUM") as dps:
              for img in range(NI):
                # ---- C1: Q,K projections (feature-major, this image) ----
                qk_i = qkp.tile([128, 2 * DT, S], BF16, tag="qk")
                v_i = vp.tile([128, ST, D], BF16, tag="v")
                if True:
                    for mi, w_sb in ((0, wq_sb), (1, wk_sb)):
                        for mt in range(DT):
                            for ch in range(QCH):
                                ps = qps.tile([128, QW], F32, name="qkps", tag="q")
                                for kt in range(DT):
                                    nc.tensor.matmul(
                                        ps[:], w_sb[:, kt, mt * 128:(mt + 1) * 128],
                                        xc[:, kt, img * S + ch * QW: img * S + (ch + 1) * QW],
                                        start=(kt == 0), stop=(kt == DT - 1))
                                nc.vector.tensor_copy(
                                    qk_i[:, mi * DT + mt, ch * QW:(ch + 1) * QW],
                                    ps[:])
                    # ---- C2: V projection (token-major, this image) ----
                    for (kt, row0, ss) in _stiles(img):
                        for ch2 in range(2):
                            ps = qps.tile([128, 384], F32, name="vps", tag="q")
                            for dti in range(DT):
                                nc.tensor.matmul(
                                    ps[:ss, :],
                                    xc[:, dti, row0:row0 + ss],
                                    wv_sb[:, dti, ch2 * 384:(ch2 + 1) * 384],
                                    start=(dti == 0), stop=(dti == DT - 1))
                            nc.vector.tensor_copy(
                                v_i[:ss, kt, ch2 * 384:(ch2 + 1) * 384],
                                ps[:ss, :])

                # ---- C3: attention (this image) ----
                hv_i = hvp.tile([128, DT, S], BF16, tag="hv")
                if True:
                    for hp_i in range(NH // 2):
                        et = hp_i
                        for qc in range(QCH):
                            e_tiles = []
                            for h01 in range(2):
                                e_t = ep.tile([128, ST, QW], BF16, tag="E",
                                              name=f"E_{h01}")
                                e_tiles.append(e_t)
                                for (kt, row0, ss) in _stiles(0):
                                    ps = sps.tile([128, QW], F32, name="scps", tag="s")
                                    nc.tensor.matmul(
                                        ps[0:ss, :],
                                        qk_i[h01 * 64:(h01 + 1) * 64, DT + et,
                                             kt * 128:kt * 128 + ss],
                                        qk_i[h01 * 64:(h01 + 1) * 64, et,
                                             qc * QW:(qc + 1) * QW],
                                        start=True, stop=True)
                                    nc.scalar.activation(
                                        e_t[0:ss, kt, :],
                                        ps[0:ss, :], AF.Exp, scale=0.125)
                            hv_ps = hps.tile([128, QW], F32)
                            d_ps = dps.tile([128, QW], F32)
                            for h01 in range(2):
                                for (kt, row0, ss) in _stiles(0):
                                    nc.tensor.matmul(
                                        hv_ps[h01 * 64:(h01 + 1) * 64, :],
                                        v_i[0:ss, kt,
                                            (2 * hp_i + h01) * 64:(2 * hp_i + h01 + 1) * 64],
                                        e_tiles[h01][0:ss, kt, :],
                                        start=(kt == 0), stop=(kt == ST - 1),
                                        tile_position=(0, 64 * h01))
                                    nc.tensor.matmul(
                                        d_ps[h01 * 64:(h01 + 1) * 64, :],
                                        ones64[0:ss, :],
                                        e_tiles[h01][0:ss, kt, :],
                                        start=(kt == 0), stop=(kt == ST - 1),
                                        tile_position=(0, 64 * h01))
                            r_sb = rsbp.tile([128, QW], F32, tag="rsb")
                            nc.vector.reciprocal_approx_fast(r_sb[:], d_ps[:])
                            nc.vector.tensor_mul(
                                hv_i[:, et, qc * QW:(qc + 1) * QW],
                                hv_ps[:], r_sb[:])

                # ---- C4: Wh + bias + residual -> nxt (this image, fp32) ----
                if True:
                    for mt in range(DT):
                        for ch in range(QCH):
                            ps = sps.tile([128, QW], F32, name="ops", tag="s")
                            for et in range(DT):
                                nc.tensor.matmul(
                                    ps[:], wh_sb[:, et, mt * 128:(mt + 1) * 128],
                                    hv_i[:, et, ch * QW:(ch + 1) * QW],
                                    start=(et == 0), stop=(et == DT - 1))
                            nc.vector.scalar_tensor_tensor(
                                nxt[:, mt, img * S + ch * QW: img * S + (ch + 1) * QW],
                                ps[:], whb_sb[:, mt:mt + 1],
                                cur[:, mt, img * S + ch * QW: img * S + (ch + 1) * QW],
                                op0=mybir.AluOpType.add, op1=mybir.AluOpType.add)

            # ---- C5: LayerNorm(nxt) -> xn (bf16) ----
            xn = xbfp.tile([128, DT, T], BF16, tag="xn")
            with tc.tile_pool(name="stps", bufs=1, space="PSUM") as stps, \
                 tc.tile_pool(name="bps", bufs=1, space="PSUM") as bps:
                for ch in range(TCH):
                    sq = xbfp.tile([128, DT, 384], BF16, tag="sq")
                    for kt in range(DT):
                        nc.vector.tensor_mul(sq[:, kt, :],
                                             nxt[:, kt, ch * 384:(ch + 1) * 384],
                                             nxt[:, kt, ch * 384:(ch + 1) * 384])
                    st0 = stps.tile([1, 384], F32, tag="st0")
                    st1 = stps.tile([1, 384], F32, tag="st1")
                    for kt in range(DT):
                        nc.tensor.matmul(st0[:], ones_f[:],
                                         nxt[:, kt, ch * 384:(ch + 1) * 384],
                                         start=(kt == 0), stop=(kt == DT - 1))
                        nc.tensor.matmul(st1[:], ones_col[:],
                                         sq[:, kt, :],
                                         start=(kt == 0), stop=(kt == DT - 1))
                    mom = smallp.tile([1, 384], F32, tag="mom")
                    nc.scalar.mul(mom[:], st0[:], 1.0 / D)
                    msq = smallp.tile([1, 384], F32, tag="msq")
                    nc.vector.tensor_mul(msq[:], mom[:], mom[:])
                    ex2 = smallp.tile([1, 384], F32, tag="ex2")
                    nc.scalar.mul(ex2[:], st1[:], 1.0 / D)
                    var = smallp.tile([1, 384], F32, tag="var")
                    nc.vector.tensor_sub(var[:], ex2[:], msq[:])
                    nc.scalar.activation(var[:], var[:], AF.Sqrt, bias=eps2[:])
                    rstd = smallp.tile([1, 384], F32, tag="rstd")
                    nc.vector.reciprocal_approx_fast(rstd[:], var[:])
                    m_ps = bps.tile([128, 384], F32, tag="mb")
                    nc.tensor.matmul(m_ps[:], ones_row[:], mom[:],
                                     start=True, stop=True)
                    r_ps = bps.tile([128, 384], F32, tag="rb")
                    nc.tensor.matmul(r_ps[:], ones_row[:], rstd[:],
                                     start=True, stop=True)
                    for mt in range(DT):
                        t_c = tmpp.tile([128, 384], F32, tag="t5a")
                        nc.vector.tensor_sub(t_c[:],
                                             nxt[:, mt, ch * 384:(ch + 1) * 384],
                                             m_ps[:])
                        t_d = tmpp.tile([128, 384], F32, tag="t5b")
                        nc.vector.tensor_mul(t_d[:], t_c[:], r_ps[:])
                        nc.scalar.activation(
                            xn[:, mt, ch * 384:(ch + 1) * 384], t_d[:],
                            AF.Identity, bias=l2b_sb[:, mt:mt + 1],
                            scale=l2s_sb[:, mt:mt + 1])

            # ---- C6: FFN + residual (in place on nxt) ----
            with tc.tile_pool(name="f2ps", bufs=1, space="PSUM") as f2ps, \
                 tc.tile_pool(name="gps", bufs=2, space="PSUM") as gps:
                for tch in range(TCH if 'ffn' not in SKIP else 0):
                    f2 = [f2ps.tile([128, 384], F32, tag=f"f2_{mt}", name=f"f2_{mt}")
                          for mt in range(DT)]
                    for ft in range(FT):
                        w1_sb = ffw.tile([128, DT, 128], BF16, tag="w1")
                        nc.sync.dma_start(w1_sb[:], w1[li, ft])
                        w2_sb = ffw.tile([128, D], BF16, tag="w2")
                        nc.sync.dma_start(w2_sb[:], w2[li, ft * 128:(ft + 1) * 128, :])
                        g_ps = gps.tile([128, 384], F32)
                        for kt in range(DT):
                            nc.tensor.matmul(
                                g_ps[:], w1_sb[:, kt, :],
                                xn[:, kt, tch * 384:(tch + 1) * 384],
                                start=(kt == 0), stop=(kt == DT - 1))
                        g_bf = gp.tile([128, 384], BF16, tag="gbf")
                        nc.scalar.activation(g_bf[:], g_ps[:], AF.Gelu,
                                             bias=b1_sb[:, ft:ft + 1])
                        for mt in range(DT):
                            nc.tensor.matmul(
                                f2[mt][:], w2_sb[:, mt * 128:(mt + 1) * 128],
                                g_bf[:], start=(ft == 0), stop=(ft == FT - 1))
                    for mt in range(DT):
                        nc.vector.scalar_tensor_tensor(
                            nxt[:, mt, tch * 384:(tch + 1) * 384],
                            f2[mt][:], b2_sb[:, mt:mt + 1],
                            nxt[:, mt, tch * 384:(tch + 1) * 384],
                            op0=mybir.AluOpType.add, op1=mybir.AluOpType.add)
            cur, nxt = nxt, cur

        # ================= Final LayerNorm -> out =================
        lnf_s = biasp.tile([128, DT], F32, tag="lnfs")
        nc.sync.dma_start(lnf_s[:], lnfs.rearrange("(t p) -> p t", p=128))
        lnf_b = biasp.tile([128, DT], F32, tag="lnfb")
        nc.sync.dma_start(lnf_b[:], lnfb.rearrange("(t p) -> p t", p=128))
        with tc.tile_pool(name="fout", bufs=2) as foutp, \
             tc.tile_pool(name="fstps", bufs=1, space="PSUM") as stps, \
             tc.tile_pool(name="fbps", bufs=1, space="PSUM") as bps:
            for ch in range(TCH):
                sqf = xbfp.tile([128, DT, 384], BF16, tag="sq")
                for kt in range(DT):
                    nc.vector.tensor_mul(sqf[:, kt, :],
                                         cur[:, kt, ch * 384:(ch + 1) * 384],
                                         cur[:, kt, ch * 384:(ch + 1) * 384])
                st0 = stps.tile([1, 384], F32, tag="st0")
                st1 = stps.tile([1, 384], F32, tag="st1")
                for kt in range(DT):
                    nc.tensor.matmul(st0[:], ones_f[:],
                                     cur[:, kt, ch * 384:(ch + 1) * 384],
                                     start=(kt == 0), stop=(kt == DT - 1))
                    nc.tensor.matmul(st1[:], ones_col[:],
                                     sqf[:, kt, :],
                                     start=(kt == 0), stop=(kt == DT - 1))
                mom = smallp.tile([1, 384], F32, tag="mom")
                nc.scalar.mul(mom[:], st0[:], 1.0 / D)
                msq = smallp.tile([1, 384], F32, tag="msq")
                nc.vector.tensor_mul(msq[:], mom[:], mom[:])
                ex2 = smallp.tile([1, 384], F32, tag="ex2")
                nc.scalar.mul(ex2[:], st1[:], 1.0 / D)
                var = smallp.tile([1, 384], F32, tag="var")
                nc.vector.tensor_sub(var[:], ex2[:], msq[:])
                nc.scalar.activation(var[:], var[:], AF.Sqrt, bias=epsf[:])
                rstd = smallp.tile([1, 384], F32, tag="rstd")
                nc.vector.reciprocal_approx_fast(rstd[:], var[:])
                m_ps = bps.tile([128, 384], F32, tag="mb")
                nc.tensor.matmul(m_ps[:], ones_row[:], mom[:],
                                 start=True, stop=True)
                r_ps = bps.tile([128, 384], F32, tag="rb")
                nc.tensor.matmul(r_ps[:], ones_row[:], rstd[:],
                                 start=True, stop=True)
                for mt in range(DT):
                    t_c = tmpp.tile([128, 384], F32, tag="t5a")
                    nc.vector.tensor_sub(t_c[:],
                                         cur[:, mt, ch * 384:(ch + 1) * 384], m_ps[:])
                    t_d = tmpp.tile([128, 384], F32, tag="t5b")
                    nc.vector.tensor_mul(t_d[:], t_c[:], r_ps[:])
                    o_sb = foutp.tile([128, 384], F32)
                    nc.scalar.activation(o_sb[:], t_d[:], AF.Identity,
                                         bias=lnf_b[:, mt:mt + 1],
                                         scale=lnf_s[:, mt:mt + 1])
                    c0 = ch * 384
                    for off in range(0, 384, 192):
                        col = c0 + off
                        img, s0 = divmod(col, S)
                        nc.sync.dma_start(
                            out[img, mt * 128:(mt + 1) * 128, s0:s0 + 192],
                            o_sb[:, off:off + 192])
    nc.finalize()
    return nc


def _pos_encoding(max_len, d):
    pos = np.arange(max_len)[:, None].astype(np.float32)
    div = np.exp(np.arange(0, d, 2).astype(np.float32) * (-np.log(10000.0) / d))
    pe = np.zeros((max_len, d), dtype=np.float32)
    pe[:, 0::2] = np.sin(pos * div)
    pe[:, 1::2] = np.cos(pos * div)
    return pe


_NC_CACHE = {}


def get_nc(n_layers=L):
    if n_layers not in _NC_CACHE:
        _NC_CACHE[n_layers] = build_kernel(n_layers)
    return _NC_CACHE[n_layers]


def make_in_maps(x, conv_w, conv_b, ln1_s, ln1_b, wq, wk, wv, wh, wh_b,
                 ln2_s, ln2_b, w1, b1, w2, b2, lnf_s, lnf_b):
    bf = ml_dtypes.bfloat16
    x = np.asarray(x, np.float32)
    patches = x.reshape(B, C, IMG // P, P, IMG // P, P)      # (B,C,ty,py,tx,px)
    patches = patches.transpose(0, 1, 3, 5, 2, 4).reshape(B, D, S).astype(bf)
    wckh = np.ascontiguousarray(
        np.asarray(conv_w, np.float32).reshape(D, D).T).astype(bf)
    pefh = np.ascontiguousarray(_pos_encoding(5000, D)[:S].T)
    shared = {
        "wck": wckh, "cb": np.asarray(conv_b, np.float32), "pef": pefh,
        "wq": np.asarray(wq, np.float32).astype(bf),
        "wk": np.asarray(wk, np.float32).astype(bf),
        "wv": np.asarray(wv, np.float32).astype(bf),
        "wh": np.asarray(wh, np.float32).astype(bf),
        "whb": np.asarray(wh_b, np.float32),
        "ln2s": np.asarray(ln2_s, np.float32),
        "ln2b": np.asarray(ln2_b, np.float32),
        "w1": np.ascontiguousarray(
            np.asarray(w1, np.float32).reshape(L, DT, 128, FT, 128)
            .transpose(0, 3, 2, 1, 4)).astype(bf),
        "b1": np.asarray(b1, np.float32),
        "w2": np.asarray(w2, np.float32).astype(bf),
        "b2": np.asarray(b2, np.float32),
        "lnfs": np.asarray(lnf_s, np.float32),
        "lnfb": np.asarray(lnf_b, np.float32),
    }
    in_maps = []
    for c in range(NCORES):
        m = dict(shared)
        m["xp"] = np.ascontiguousarray(patches[c * NI:(c + 1) * NI])
        in_maps.append(m)
    return in_maps


def assemble_output(results):
    out = np.empty((B, S, D), np.float32)
    for c in range(NCORES):
        o = results[c]["out"]
        for i in range(NI):
            out[c * NI + i] = o[i].T
    return out


def kernel(**inputs) -> np.ndarray:
    nc = get_nc()
    in_maps = make_in_maps(**inputs)
    res = run_bass_kernel_spmd(nc, in_maps, core_ids=list(range(NCORES)))
    return assemble_output(res.results)

